# revision 14
# baseline (speedup 1.0000x reference)
"""Trainium2 Bass kernel for nn_BasicBlock_Q (quantized BasicBlock, dense CNN).

Computation (see the module's reference):
    wq1 = dorefa_quant(w1) * pat1 ; out = conv3x3(x, wq1)
    out = act_quant(batchnorm(out, g1, b1))          # 4-bit act quant
    wq2 = dorefa_quant(w2) * pat2 ; out = conv3x3(out, wq2)
    out = batchnorm(out, g2, b2) + x ; out = act_quant(out)

Distribution: data-parallel over the batch (2048 -> 8 cores x 256 images).
BatchNorm uses full-batch statistics, so each BN does a tiny (1 KB)
cross-core AllReduce of per-channel (mean, E[x^2]).

Numerical scheme (all matmul operands are exactly representable):
  - quantized weights are stored as integers (2k-15) in bf16 (exact),
    the 1/15 scales are folded into the BN affine transforms.
  - conv1 splits fp32 x into bf16 hi+lo and accumulates both passes in
    PSUM (error ~4e-6 relative, validated: final L2 rel err ~1e-3 vs
    fp32 reference, from inevitable quantization-boundary flips).
  - conv2's inputs are the quantized activations as integers 0..15 in
    bf16, so conv2 is exact integer arithmetic.
  - round() is implemented as (x + 2^23) - 2^23 (exact round-half-even
    in fp32, matching jnp.round).
  - 3x3 "same" conv: inputs live in SBUF in a zero-padded 10x10 per-image
    layout; each tap is one shifted strided read, accumulated over 9 taps
    into one PSUM bank (contiguous [64, 512] output per chunk).

Layout per core: [128 partitions = 2 groups x 64 channels]. The two
groups' matmuls use disjoint PE-array quadrants (tile_position (0,0) /
(64,64)) and run concurrently.
"""

import sys

for _p in ("/opt/trn_rl_repo",):
    if _p not in sys.path:
        sys.path.insert(0, _p)

import numpy as np

# ---- problem geometry (hardcoded from the problem spec) ----
B, CH, H, W = 2048, 64, 8, 8
NCORES = 8
PIX = H * W  # 64
PH, PW = H + 2, W + 2
PPIX = PH * PW  # 100, padded image size

MAGIC = float(2.0**23)
EPS = 1e-5

TRACE = False  # set by test.py for profiling runs
TRACE_KWARGS = {}
LAST_RESULTS = None


def _build(nc, img_per_group, nchunk, dma_slabs=4):
    """Emit the Tile program for one core processing 2*img_per_group images."""
    import concourse.bass as bass
    import concourse.tile as tile
    from concourse import mybir
    from concourse.tile import TileContext
    from contextlib import ExitStack

    dt = mybir.dt
    Alu = mybir.AluOpType
    Act = mybir.ActivationFunctionType

    G = 2
    IPG = img_per_group            # images per partition-group
    FREE = IPG * PIX               # free size of the compact buffers
    PFREE = IPG * PPIX             # free size of the padded buffers
    IPC = IPG // nchunk            # images per chunk
    CHF = IPC * PIX                # chunk free size (<=512 for one PSUM bank)
    PCHF = IPC * PPIX
    assert CHF <= 512
    dma_slabs = min(dma_slabs, nchunk)
    SLAB = nchunk // dma_slabs     # chunks per IO slab
    assert dma_slabs * SLAB == nchunk

    pb = G * IPG                   # images per core

    # ---- DRAM I/O ----
    x_d = nc.dram_tensor("x", [pb, CH, H, W], dt.float32, kind="ExternalInput")
    w1_d = nc.dram_tensor("w1", [CH, CH, 3, 3], dt.float32, kind="ExternalInput")
    w2_d = nc.dram_tensor("w2", [CH, CH, 3, 3], dt.float32, kind="ExternalInput")
    p1_d = nc.dram_tensor("pat1", [CH, CH, 3, 3], dt.float32, kind="ExternalInput")
    p2_d = nc.dram_tensor("pat2", [CH, CH, 3, 3], dt.float32, kind="ExternalInput")
    g1_d = nc.dram_tensor("gamma1", [CH], dt.float32, kind="ExternalInput")
    b1_d = nc.dram_tensor("beta1", [CH], dt.float32, kind="ExternalInput")
    g2_d = nc.dram_tensor("gamma2", [CH], dt.float32, kind="ExternalInput")
    b2_d = nc.dram_tensor("beta2", [CH], dt.float32, kind="ExternalInput")
    out_d = nc.dram_tensor("out", [pb, CH, H, W], dt.float32, kind="ExternalOutput")

    with ExitStack() as ctx:
        tc = ctx.enter_context(TileContext(nc))

        big = ctx.enter_context(tc.tile_pool(name="big", bufs=1))
        wp = ctx.enter_context(tc.tile_pool(name="wp", bufs=1))
        work = ctx.enter_context(tc.tile_pool(name="work", bufs=2))
        psA_pool = ctx.enter_context(tc.tile_pool(name="psA", bufs=2, space="PSUM"))
        psB_pool = ctx.enter_context(tc.tile_pool(name="psB", bufs=2, space="PSUM"))
        smalls = ctx.enter_context(tc.tile_pool(name="smalls", bufs=1))
        dram = ctx.enter_context(tc.tile_pool(name="dram", bufs=1, space="DRAM"))

        # ---- persistent SBUF tensors ----
        xpad = big.tile([128, PFREE], dt.float32, tag="xpad")   # zero-padded 10x10 images
        out1 = big.tile([128, FREE], dt.float32, tag="out1")    # conv1 acc; reused for final out
        rbuf = big.tile([128, PFREE], dt.bfloat16, tag="rbuf")  # padded quantized act1 ints 0..15
        out2 = big.tile([128, FREE], dt.float32, tag="out2")    # conv2 acc (integer valued)

        wq1 = wp.tile([128, 9 * CH], dt.bfloat16, tag="wq1")    # [cin, tap, cout] integer weights
        wq2 = wp.tile([128, 9 * CH], dt.bfloat16, tag="wq2")

        stats1 = smalls.tile([128, nchunk * 6], dt.float32, tag="stats1")
        stats2 = smalls.tile([128, nchunk * 6], dt.float32, tag="stats2")
        aff1 = smalls.tile([128, 2], dt.float32, tag="aff1")    # col0 scale, col1 bias
        aff2 = smalls.tile([128, 2], dt.float32, tag="aff2")
        # gamma/beta as 4 separate first-touch tiles (keeps their loads waitless)
        gbt = [
            smalls.tile([64, 1], dt.float32, tag=f"gb{i}", name=f"gb{i}")
            for i in range(4)
        ]

        # padded [p, img, 10, 10] and compact [p, img, 64] views
        pv = lambda t: t[:].rearrange("p (i r c) -> p i r c", r=PH, c=PW)
        cv = lambda t: t[:].rearrange("p (i q) -> p i q", q=PIX)

        # ---- zero the padded buffers (borders must stay zero) ----
        nc.vector.memset(xpad[:], 0.0)
        nc.vector.memset(rbuf[:], 0.0)

        # ---- load gamma/beta (first-touch sync DMAs) ----
        for col, t_d in enumerate((g1_d, b1_d, g2_d, b2_d)):
            nc.sync.dma_start(gbt[col][:], t_d.ap().rearrange("(c o) -> c o", o=1))

        # ---- load x compact into out1 (staging), then DVE-copy into the
        # padded interior.  sync DMAs here are first-touch (<=1 wait);
        # the strided 4-dim scatter runs on DVE where APs are unrestricted.
        for g in range(2):
            src = x_d.ap()[g * IPG : (g + 1) * IPG].rearrange("i c h w -> c i (h w)")
            nc.sync.dma_start(cv(out1)[64 * g : 64 * g + 64, :, :], src)
        for s in range(dma_slabs):
            i0, i1 = s * (IPG // dma_slabs), (s + 1) * (IPG // dma_slabs)
            for g in range(2):
                pg = slice(64 * g, 64 * g + 64)
                nc.vector.tensor_copy(
                    pv(xpad)[pg, i0:i1, 1 : 1 + H, 1 : 1 + W],
                    cv(out1)[pg, i0:i1, :].rearrange("p i (h w) -> p i h w", w=W),
                )

        # ---- weight prep: integer DoReFa weights, masked ----
        def prep_weights(wt, pt, wq_tile, tags):
            # tanh via degree-11 odd Taylor poly (|w| < ~0.3, err < 1e-8)
            x2 = work.tile([128, 576], dt.float32, tag=tags[0], name="prep_x2")
            p = work.tile([128, 576], dt.float32, tag=tags[1], name="prep_p")
            t = work.tile([128, 576], dt.float32, tag=tags[2], name="prep_t")
            nc.vector.tensor_tensor(x2[:], wt[:], wt[:], Alu.mult)
            nc.vector.tensor_scalar(
                p[:], x2[:], float(-1382.0 / 155925.0), float(62.0 / 2835.0), Alu.mult, Alu.add
            )
            for c in (-17.0 / 315.0, 2.0 / 15.0, -1.0 / 3.0):
                nc.vector.tensor_tensor(p[:], p[:], x2[:], Alu.mult)
                nc.vector.tensor_scalar(p[:], p[:], float(c), None, Alu.add)
            nc.vector.tensor_tensor(t[:], wt[:], x2[:], Alu.mult)   # w*x2
            nc.vector.tensor_tensor(t[:], t[:], p[:], Alu.mult)     # (w*x2)*p
            nc.vector.tensor_tensor(t[:], t[:], wt[:], Alu.add)     # + w  -> tanh(w)
            # global absmax over all weights: free-dim reduce, DMA transpose
            # to one partition, reduce again, broadcast scale via K=1 matmul.
            mx = smalls.tile([128, 1], dt.float32, tag="wprep_mx", name="wprep_mx")
            nc.vector.reduce_max(
                mx[:], t[:], axis=mybir.AxisListType.X, apply_absolute_value=True
            )
            mxT = smalls.tile([1, 128], dt.float32, tag="wprep_mxT", name="wprep_mxT")
            nc.gpsimd.dma_start(mxT[0:1, :], mx[:])
            grec = smalls.tile([1, 1], dt.float32, tag="wprep_grec", name="wprep_grec")
            nc.vector.reduce_max(grec[0:1, 0:1], mxT[0:1, :], axis=mybir.AxisListType.X)
            nc.vector.reciprocal(grec[0:1, 0:1], grec[0:1, 0:1])
            nc.vector.tensor_scalar(
                grec[0:1, 0:1], grec[0:1, 0:1], 7.5, None, Alu.mult
            )  # 15/(2M)
            # replicate the scalar along partition 0's free dim, then a
            # partition-scatter DMA broadcasts it to [128, 1].
            srow = smalls.tile([1, 128], dt.float32, tag="wprep_srow", name="wprep_srow")
            nc.vector.memset(srow[0:1, :], 1.0)
            nc.vector.tensor_scalar(
                srow[0:1, :], srow[0:1, :], grec[0:1, 0:1], None, Alu.mult
            )
            rec = smalls.tile([128, 1], dt.float32, tag="wprep_rec", name="wprep_rec")
            nc.gpsimd.dma_start(rec[:, 0:1], srow[0:1, :])
            # u = t*s + 7.5 in [0,15]; q = round(u); wi = 2q-15; *= mask
            nc.vector.tensor_scalar(t[:], t[:], rec[:, 0:1], 7.5, Alu.mult, Alu.add)
            nc.vector.tensor_scalar(t[:], t[:], MAGIC, MAGIC, Alu.add, Alu.subtract)
            nc.vector.tensor_scalar(t[:], t[:], 2.0, 15.0, Alu.mult, Alu.subtract)
            nc.vector.tensor_tensor(wq_tile[:], t[:], pt[:], Alu.mult)

        # raw weight/mask loads: dedicated first-touch tiles, permuted to
        # [cin, tap, cout] with both partition-half copies.
        raw = {}
        for nm, t_d in (("w1", w1_d), ("p1", p1_d), ("w2", w2_d), ("p2", p2_d)):
            rt = wp.tile([128, 576], dt.float32, tag="raw" + nm, name="raw" + nm)
            src = t_d.ap().rearrange("o i kh kw -> i (kh kw) o")
            rv = rt[:].rearrange("p (t o) -> p t o", o=CH)
            for g in range(2):
                nc.sync.dma_start(rv[64 * g : 64 * g + 64], src)
            raw[nm] = rt

        prep_weights(raw["w1"], raw["p1"], wq1, ("st2u", "st2c", "st4q"))
        prep_weights(raw["w2"], raw["p2"], wq2, ("st2u", "st2c", "st4q"))

        # ---- conv: 9 shifted taps over padded input, 2 concurrent PE quadrants ----
        def conv_chunk(j, wq_tile, rhs_tiles, rhs_off, psA, psB):
            """rhs_tiles: list of padded bf16 tiles; rhs_off: image offset of chunk j
            inside those tiles. Accumulates into psA[0:64], psB[64:128]."""
            wv = wq_tile[:].rearrange("p (t o) -> p t o", o=CH)
            npass = len(rhs_tiles)
            for pi, rt in enumerate(rhs_tiles):
                rv = pv(rt)
                for ky in range(3):
                    for kx in range(3):
                        t = ky * 3 + kx
                        first = pi == 0 and t == 0
                        last = pi == npass - 1 and t == 8
                        for g, ps in ((0, psA), (1, psB)):
                            pg = 64 * g
                            nc.tensor.matmul(
                                ps[pg : pg + 64, :CHF],
                                wv[pg : pg + 64, t, :],
                                rv[pg : pg + 64, rhs_off : rhs_off + IPC,
                                   ky : ky + H, kx : kx + W],
                                start=first,
                                stop=last,
                                skip_group_check=True,
                            )

        def epilogue_chunk(j, psA, psB, acc, stats):
            sl = slice(j * CHF, (j + 1) * CHF)
            sv = stats[:].rearrange("p (c s) -> p c s", s=6)
            for g, ps in ((0, psA), (1, psB)):
                pg = slice(64 * g, 64 * g + 64)
                nc.scalar.activation(acc[pg, sl], ps[pg, :CHF], Act.Identity)
                nc.vector.bn_stats(sv[pg, j, :], ps[pg, :CHF])

        # ---- BN affine computation (stats -> per-channel scale/bias) ----
        def bn_affine(stats, aff, gcol, bcol, eps_scaled, scale15, tagp):
            T = lambda n, s=[128, 1]: smalls.tile(
                s, dt.float32, tag=tagp + n, name=tagp + n
            )
            aggr = T("aggr", [128, 2])
            nc.vector.bn_aggr(aggr[:], stats[:].rearrange("p (c s) -> p c s", s=6))
            arin = T("arin", [128, 2])
            m2 = T("m2")
            nc.vector.tensor_tensor(m2[:], aggr[:, 0:1], aggr[:, 0:1], Alu.mult)
            nc.vector.tensor_copy(arin[:, 0:1], aggr[:, 0:1])
            nc.vector.tensor_tensor(arin[:, 1:2], aggr[:, 1:2], m2[:], Alu.add)
            ccin = dram.tile([128, 2], dt.float32, tag=tagp + "ccin", name=tagp + "ccin")
            ccout = dram.tile(
                [128, 2], dt.float32, tag=tagp + "ccout", name=tagp + "ccout"
            )
            nc.gpsimd.dma_start(ccin[:], arin[:])
            nc.gpsimd.collective_compute(
                "AllReduce",
                Alu.add,
                replica_groups=[list(range(NCORES))],
                ins=[ccin.opt()],
                outs=[ccout.opt()],
            )
            arout = T("arout", [128, 2])
            nc.gpsimd.dma_start(arout[:], ccout[:])
            gB = T("gB", [128, 2])
            nc.gpsimd.dma_start(gB[0:64, :], arout[64:128, :])
            s16 = T("s16", [128, 2])
            nc.vector.tensor_tensor(s16[0:64, :], arout[0:64, :], gB[0:64, :], Alu.add)
            mI = T("mI")
            e2 = T("e2")
            nc.vector.tensor_scalar(mI[0:64], s16[0:64, 0:1], 1.0 / 16.0, None, Alu.mult)
            nc.vector.tensor_scalar(e2[0:64], s16[0:64, 1:2], 1.0 / 16.0, None, Alu.mult)
            vI = T("vI")
            nc.vector.tensor_tensor(vI[0:64], mI[0:64], mI[0:64], Alu.mult)
            nc.vector.tensor_tensor(vI[0:64], e2[0:64], vI[0:64], Alu.subtract)
            nc.vector.tensor_scalar(vI[0:64], vI[0:64], float(eps_scaled), None, Alu.add)
            rc = T("rc")
            nc.vector.reciprocal(rc[0:64], vI[0:64])
            rs = T("rs")
            nc.scalar.activation(rs[0:64], rc[0:64], Act.Sqrt)  # rsqrt(var+eps)
            sg = T("sg")
            nc.vector.tensor_tensor(sg[0:64], rs[0:64], gbt[gcol][:], Alu.mult)
            if scale15:
                nc.vector.tensor_scalar(sg[0:64], sg[0:64], 15.0, None, Alu.mult)
            bb = T("bb")
            nc.vector.tensor_scalar(
                bb[0:64], gbt[bcol][:], 15.0 if scale15 else 1.0, None, Alu.mult
            )
            ms = T("ms")
            nc.vector.tensor_tensor(ms[0:64], mI[0:64], sg[0:64], Alu.mult)
            nc.vector.tensor_copy(aff[0:64, 0:1], sg[0:64])
            nc.vector.tensor_tensor(aff[0:64, 1:2], bb[0:64], ms[0:64], Alu.subtract)
            nc.gpsimd.dma_start(aff[64:128, :], aff[0:64, :])

        # ---- phase 1: conv1 (bf16 hi + lo passes, hi/lo built per chunk) ----
        for j in range(nchunk):
            hip = work.tile([128, PCHF], dt.bfloat16, tag="hip", name="hip")
            lop = work.tile([128, PCHF], dt.bfloat16, tag="lop", name="lop")
            sl = slice(j * PCHF, (j + 1) * PCHF)
            nc.vector.tensor_copy(hip[:, :PCHF], xpad[:, sl])
            nc.vector.tensor_tensor(lop[:, :PCHF], xpad[:, sl], hip[:, :PCHF], Alu.subtract)
            psA = psA_pool.tile([128, 512], dt.float32, tag="psA", name="psA")
            psB = psB_pool.tile([128, 512], dt.float32, tag="psB", name="psB")
            conv_chunk(j, wq1, [hip, lop], 0, psA, psB)
            epilogue_chunk(j, psA, psB, out1, stats1)

        bn_affine(stats1, aff1, 0, 1, 225.0 * EPS, True, "bn1")

        # ---- phase 2: act-quant (r = clip(round(aff(out1)),0,15)) + conv2 ----
        for j in range(nchunk):
            sl = slice(j * CHF, (j + 1) * CHF)
            u = work.tile([128, 512], dt.float32, tag="st2u", name="u2")
            c = work.tile([128, 512], dt.float32, tag="st2c", name="c2")
            nc.scalar.activation(
                u[:, :CHF], out1[:, sl], Act.Identity,
                bias=aff1[:, 1:2], scale=aff1[:, 0:1],
            )
            nc.gpsimd.tensor_scalar(c[:, :CHF], u[:, :CHF], 15.0, 0.0, Alu.min, Alu.max)
            nc.vector.tensor_scalar(
                pv(rbuf)[:, j * IPC : (j + 1) * IPC, 1 : 1 + H, 1 : 1 + W],
                cv(c)[:, :IPC, :],
                MAGIC, MAGIC, Alu.add, Alu.subtract,
            )
            psA = psA_pool.tile([128, 512], dt.float32, tag="psA", name="psA")
            psB = psB_pool.tile([128, 512], dt.float32, tag="psB", name="psB")
            conv_chunk(j, wq2, [rbuf], j * IPC, psA, psB)
            epilogue_chunk(j, psA, psB, out2, stats2)

        bn_affine(stats2, aff2, 2, 3, 225.0 * 225.0 * EPS, False, "bn2")

        # ---- phase 3: final = round(clip((aff(out2)+x)*15,0,15))/15 ----
        for j in range(nchunk):
            sl = slice(j * CHF, (j + 1) * CHF)
            u = work.tile([128, 512], dt.float32, tag="st4u", name="u4")
            v = work.tile([128, 512], dt.float32, tag="st4v", name="v4")
            q = work.tile([128, 512], dt.float32, tag="st4q", name="q4")
            nc.scalar.activation(
                u[:, :CHF], out2[:, sl], Act.Identity,
                bias=aff2[:, 1:2], scale=aff2[:, 0:1],
            )
            nc.vector.tensor_tensor(
                cv(v)[:, :IPC, :],
                cv(u)[:, :IPC, :],
                pv(xpad)[:, j * IPC : (j + 1) * IPC, 1 : 1 + H, 1 : 1 + W],
                Alu.add,
            )
            nc.gpsimd.tensor_scalar(q[:, :CHF], v[:, :CHF], 15.0, 15.0, Alu.mult, Alu.min)
            nc.vector.tensor_scalar(q[:, :CHF], q[:, :CHF], 0.0, MAGIC, Alu.max, Alu.add)
            # write final values into out1 (free after phase 2)
            nc.gpsimd.tensor_scalar(
                out1[:, sl], q[:, :CHF], MAGIC, 1.0 / 15.0, Alu.subtract, Alu.mult
            )
            if (j + 1) % SLAB == 0:
                i0, i1 = (j + 1 - SLAB) * IPC, (j + 1) * IPC
                for g in range(2):
                    dst = out_d.ap()[g * IPG + i0 : g * IPG + i1].rearrange(
                        "i c h w -> c i (h w)"
                    )
                    nc.gpsimd.dma_start(dst, cv(out1)[64 * g : 64 * g + 64, i0:i1, :])

    return nc


_CACHE = {}


def _get_nc(img_per_group, nchunk):
    key = (img_per_group, nchunk)
    if key not in _CACHE:
        from concourse import bacc

        nc = bacc.Bacc(
            "TRN2", target_bir_lowering=False, debug=False, num_devices=NCORES
        )
        _build(nc, img_per_group, nchunk)
        nc.compile()
        _CACHE[key] = nc
    return _CACHE[key]


def kernel(**inputs):
    global LAST_RESULTS
    from concourse.bass_utils import run_bass_kernel_spmd

    x = np.ascontiguousarray(np.asarray(inputs["x"], dtype=np.float32))
    pb = x.shape[0] // NCORES
    nc = _get_nc(pb // 2, max(1, (pb // 2 * PIX) // 512))

    shared = {
        k: np.ascontiguousarray(np.asarray(inputs[k], dtype=np.float32))
        for k in ("w1", "w2", "pat1", "pat2", "gamma1", "beta1", "gamma2", "beta2")
    }
    in_maps = [{"x": x[c * pb : (c + 1) * pb], **shared} for c in range(NCORES)]
    res = run_bass_kernel_spmd(
        nc, in_maps, core_ids=list(range(NCORES)), trace=TRACE, **TRACE_KWARGS
    )
    LAST_RESULTS = res
    out = np.concatenate([res.results[c]["out"] for c in range(NCORES)], axis=0)
    return out.astype(np.float32)
